# revision 5
# baseline (speedup 1.0000x reference)
"""AudioAttention forward on 8 Trainium2 NeuronCores (Bass/Tile).

Reference computation (eval-mode AudioAttention):
    z      = mean_pool(Z_img)                    # [B, C]
    z_img, query = z[:, :C-A], z[:, C-A:]
    snd    = Z_snd[pad_idx]                      # [G, S, C] ragged gather
    value, key = snd[..., :C-A], snd[..., C-A:]
    scores = query @ key^T  (per group), masked softmax over S
    M_snd  = attn @ value                        # [G, B, C-A]
    M_img  = broadcast(z_img)                    # [G, B, C-A]

Sharding: groups sorted by size, dealt round-robin to 8 cores -> one
SPMD program serves all cores. Slot capacities are the per-slot max
size rounded up to 64: token chunks of 128 may span slot boundaries;
per-slot accumulation uses partition-sliced matmuls (bases in {0,64}).

DMA: each dma_start costs one descriptor per SBUF partition line and
the HWDGE ring generates descriptors serially at ~45ns each before the
doorbell, so descriptor COUNT (not bytes) sets latency. Hence: one
keys DMA (65 desc, query folded into cols 0:16), two value slices
(128 desc each), 4x16-desc output stores. Values travel as fp8e3
(e3m4) which halves value bytes; keys/attn stay fp16.

Tensor engine: the per-slot accumulation m_j[16,450] uses only 16 of
128 PE weight columns, so 4 slots run CONCURRENTLY via column tiling:
slot j accumulates at PSUM partitions 32*(j%4)..+16 of a shared
[128,450] tile (tile_position=(base, 32*(j%4)) auto-derived). One
128-lane copy evacuates 4 slots at once to SBUF. No on-device divide:
the denominator column ships with the output and the host divides.

Device kernel per 128-token chunk k:
  scoresT [128,B] = matmul(lhsT=keyT_ext[65,128], rhs=keyT_ext[:,0:16])
      row 64 of keyT_ext is (-shift) for valid tokens / -30000 for
      padding; col 0:16 row 64 is ones -> mask+shift folded into the
      contraction (exp(-30000) == 0 exactly).
  attnT = exp(scoresT)            (ACT, PSUM -> SBUF fp16)
  per slot piece (rows a:b):
    m4[32q:32q+16] += matmul(lhsT=attnT[a:b,k], rhs=val[a:b,chunk k])
      val column 448 is 1.0 for valid rows -> denominator column.
"""

import sys

if "/opt/trn_rl_repo" not in sys.path:
    sys.path.insert(0, "/opt/trn_rl_repo")

import numpy as np
import ml_dtypes

N_CORES = 8
CHUNK = 128
ALIGN = 64          # slot capacity alignment (matmul base_partition in {0,64})
VAL_FP8 = True      # values as float8e3 (e3m4); False -> fp16
GEXP = 8            # chunks per exp batch
N_WARM = 10         # PE warm-up matmuls (HAM un-throttle)
COLT = 4            # column-tiling ways (slots per PSUM round)

LAST_RESULTS = None  # BassKernelResults of the most recent run (for test harness)


def _plan(caps):
    """Per slot, list of (chunk, a, b) partition-sliced matmul pieces."""
    pieces = []
    o = 0
    for cap in caps:
        sl = []
        lo = o
        while lo < o + cap:
            k = lo // CHUNK
            hi = min(o + cap, (k + 1) * CHUNK)
            sl.append((k, lo - k * CHUNK, hi - k * CHUNK))
            lo = hi
        pieces.append(sl)
        o += cap
    return pieces


def _build_program(caps, gpc, ca):
    from concourse import bacc, mybir
    from concourse.tile import TileContext

    vw = ca + 2  # value row width: features + denominator + pad-to-even
    sum_caps = int(sum(caps))
    n_chunks = sum_caps // CHUNK
    assert sum_caps % CHUNK == 0 and gpc % COLT == 0
    rounds = gpc // COLT
    nc = bacc.Bacc(None, target_bir_lowering=False, debug=False)

    f32 = mybir.dt.float32
    f16 = mybir.dt.float16
    bf16 = mybir.dt.bfloat16
    vdt = mybir.dt.float8e3 if VAL_FP8 else f16
    kc = 16 + sum_caps  # query cols 0:16, then keys
    keys_d = nc.dram_tensor("keysT", [65, kc], f16, kind="ExternalInput")
    vals_d = nc.dram_tensor("vals", [CHUNK, n_chunks * vw], vdt, kind="ExternalInput")
    # group-major output: group q (partitions 32q..32q+16) owns slots
    # j%COLT==q, laid out round-major within the group
    out_d = nc.dram_tensor("out", [16, gpc * vw], f16, kind="ExternalOutput")

    pieces = _plan(caps)

    def vsplit(n, parts):
        q, r = divmod(n, parts)
        out, a = [], 0
        for i in range(parts):
            b = a + q + (1 if i < r else 0)
            if b > a:
                out.append((a, b))
            a = b
        return out

    with TileContext(nc) as tc:
        with (
            tc.tile_pool(name="resid", bufs=1) as rpool,
            tc.tile_pool(name="scps", bufs=3, space="PSUM") as scpsum,
            tc.tile_pool(name="mps", bufs=3, space="PSUM") as mpsum,
            tc.tile_pool(name="wps", bufs=1, space="PSUM") as wpsum,
        ):
            ktile = rpool.tile([65, kc], f16)
            vtile = rpool.tile([CHUNK, n_chunks * vw], vdt)
            # Keys gate the whole score pipeline: split them across BOTH
            # rings as the first transfer on each (descriptor generation is
            # serial per ring at ~45ns/desc, so nothing may sit ahead of
            # them). Value slices follow, one per ring.
            vsl = vsplit(n_chunks, 2)
            kmid = 16 + vsl[0][1] * CHUNK  # keys for chunks of slice 0 + query
            nc.scalar.dma_start(out=ktile[:, :kmid], in_=keys_d[:, :kmid])
            nc.sync.dma_start(out=ktile[:, kmid:], in_=keys_d[:, kmid:])
            nc.sync.dma_start(
                out=vtile[:, : vsl[0][1] * vw], in_=vals_d[:, : vsl[0][1] * vw]
            )
            nc.scalar.dma_start(
                out=vtile[:, vsl[1][0] * vw :], in_=vals_d[:, vsl[1][0] * vw :]
            )
            obuf = rpool.tile([CHUNK, rounds * vw], f16)

            warm = rpool.tile([CHUNK, 512], bf16)
            nc.vector.memset(warm[:], 0.0)
            wps = wpsum.tile([CHUNK, 512], f32)
            for _ in range(N_WARM):
                nc.tensor.matmul(wps[:], warm[:, :CHUNK], warm[:], start=True, stop=True)

            # Eager scores + exp for every chunk; attn resident in SBUF.
            attn = rpool.tile([CHUNK, n_chunks * 16], f16)
            n_batches = -(-n_chunks // GEXP)
            for gi in range(n_batches):
                n = min(GEXP, n_chunks - gi * GEXP)
                sc = scpsum.tile([CHUNK, n * 16], f32, name=f"sc{gi}", tag="sc")
                for x in range(n):
                    t0 = 16 + (gi * GEXP + x) * CHUNK
                    nc.tensor.matmul(
                        sc[:, x * 16 : (x + 1) * 16],
                        ktile[:, t0 : t0 + CHUNK],
                        ktile[:, 0:16],
                        start=True,
                        stop=True,
                    )
                nc.scalar.activation(
                    attn[:, gi * GEXP * 16 : (gi * GEXP + n) * 16],
                    sc[:],
                    mybir.ActivationFunctionType.Exp,
                )

            # Per round: COLT slots accumulate concurrently in one PSUM tile
            # (column tiling), then one 128-lane copy evacuates all of them.
            for r in range(rounds):
                m4 = mpsum.tile([CHUNK, vw], f32, name=f"m{r}", tag="m")
                for q in range(COLT):
                    j = r * COLT + q
                    sl = pieces[j]
                    for pi, (k, a, b) in enumerate(sl):
                        nc.tensor.matmul(
                            m4[32 * q : 32 * q + 16, :],
                            attn[a:b, k * 16 : (k + 1) * 16],
                            vtile[a:b, k * vw : (k + 1) * vw],
                            start=(pi == 0),
                            stop=(pi == len(sl) - 1),
                            # base partition 96 trips the auto-derive assert;
                            # positions are the operands' bases anyway
                            tile_position=(a if b - a <= 64 else 0, 32 * q),
                        )
                dst = obuf[:, r * vw : (r + 1) * vw]
                if r % 2 == 0:
                    nc.vector.tensor_copy(dst, m4[:])
                else:
                    nc.scalar.activation(
                        dst, m4[:], mybir.ActivationFunctionType.Copy
                    )

            engs = [nc.sync, nc.scalar]
            for q in range(COLT):
                engs[q % 2].dma_start(
                    out=out_d[:, q * rounds * vw : (q + 1) * rounds * vw],
                    in_=obuf[32 * q : 32 * q + 16, :],
                )

    nc.finalize()
    return nc


def kernel(Z_img, Z_snd, pad_idx, pad_mask, attn_dims):
    global LAST_RESULTS
    import os

    from concourse.bass_utils import run_bass_kernel_spmd

    Z_img = np.asarray(Z_img, dtype=np.float32)
    Z_snd = np.asarray(Z_snd, dtype=np.float32)
    pad_idx = np.asarray(pad_idx)
    pad_mask = np.asarray(pad_mask).astype(bool)
    A = int(attn_dims)

    B = Z_img.shape[0]
    C = Z_img.shape[1]
    CA = C - A
    G = pad_idx.shape[0]
    assert B == 16 and G % (N_CORES * COLT) == 0, (B, G)
    gpc = G // N_CORES
    rounds = gpc // COLT

    z = Z_img.reshape(B, C, -1).mean(axis=2)
    z_img, query = z[:, :CA], z[:, CA:]

    sizes = pad_mask.sum(axis=1).astype(np.int64)
    order = np.argsort(-sizes, kind="stable")  # group ids, size descending
    caps = -(-np.maximum(sizes[order[0::N_CORES]], 1) // ALIGN) * ALIGN
    if caps.sum() % CHUNK:
        caps[-1] += CHUNK - caps.sum() % CHUNK
    sum_caps = int(caps.sum())
    n_chunks = sum_caps // CHUNK
    slot_off = np.concatenate([[0], np.cumsum(caps)[:-1]]).astype(np.int64)

    q_norm_max = float(np.linalg.norm(query, axis=1).max())
    vw = CA + 2
    vdt = ml_dtypes.float8_e3m4 if VAL_FP8 else np.float16

    in_maps = []
    for c in range(N_CORES):
        keysT = np.zeros((65, 16 + sum_caps), dtype=np.float32)
        keysT[:64, 0:16] = query.T
        keysT[64, 0:16] = 1.0
        keysT[64, 16:] = -30000.0  # pad columns -> exp == 0 exactly
        vals = np.zeros((sum_caps, vw), dtype=np.float32)
        for j in range(gpc):
            g = int(order[j * N_CORES + c])
            s = int(sizes[g])
            o = int(slot_off[j])
            if s == 0:
                keysT[64, 16 + o] = 0.0
                vals[o, CA] = 1.0
                continue
            idx = pad_idx[g][pad_mask[g]]
            rows = Z_snd[idx]
            keysT[:64, 16 + o : 16 + o + s] = rows[:, CA:].T
            k_norm_max = float(np.linalg.norm(rows[:, CA:], axis=1).max())
            shift = min(q_norm_max * k_norm_max, 80.0)
            keysT[64, 16 + o : 16 + o + s] = -shift
            vals[o : o + s, :CA] = rows[:, :CA]
            vals[o : o + s, CA] = 1.0
        vimg = np.ascontiguousarray(
            vals.reshape(n_chunks, CHUNK, vw).transpose(1, 0, 2)
        ).reshape(CHUNK, n_chunks * vw).astype(vdt)
        in_maps.append({"keysT": keysT.astype(np.float16), "vals": vimg})

    nc = _build_program(caps, gpc, CA)
    trace = bool(os.environ.get("AUDIOATTN_TRACE"))
    res = run_bass_kernel_spmd(
        nc, in_maps, list(range(N_CORES)), trace=trace,
        tmpdir=os.environ.get("AUDIOATTN_TRACE_DIR") if trace else None,
    )
    LAST_RESULTS = res

    M_snd = np.empty((G, B, CA), dtype=np.float32)
    for c in range(N_CORES):
        # out layout: [16, (q * rounds + r) * vw + col], slot j = r*COLT + q
        out_c = (
            res.results[c]["out"].astype(np.float32).reshape(B, COLT, rounds, vw)
        )
        num = out_c[..., :CA]
        den = out_c[..., CA : CA + 1]
        mm = num / den  # [B, COLT, rounds, CA]
        for j in range(gpc):
            M_snd[order[j * N_CORES + c]] = mm[:, j % COLT, j // COLT]

    M_img = np.broadcast_to(z_img[None], (G, B, CA))
    return M_img, M_snd


# revision 6
# speedup vs baseline: 1.0644x; 1.0644x over previous
"""AudioAttention forward on 8 Trainium2 NeuronCores (Bass/Tile).

Reference computation (eval-mode AudioAttention):
    z      = mean_pool(Z_img)                    # [B, C]
    z_img, query = z[:, :C-A], z[:, C-A:]
    snd    = Z_snd[pad_idx]                      # [G, S, C] ragged gather
    value, key = snd[..., :C-A], snd[..., C-A:]
    scores = query @ key^T  (per group), masked softmax over S
    M_snd  = attn @ value                        # [G, B, C-A]
    M_img  = broadcast(z_img)                    # [G, B, C-A]

Sharding: groups sorted by size, dealt round-robin to 8 cores -> one
SPMD program serves all cores. Slot capacities are the per-slot max
size rounded up to 64: token chunks of 128 may span slot boundaries;
per-slot accumulation uses partition-sliced matmuls (bases in {0,64}).

DMA: each dma_start costs one descriptor per SBUF partition line and
the HWDGE ring generates descriptors serially at ~45ns each before the
doorbell, so descriptor COUNT (not bytes) sets latency. Hence: one
keys DMA (65 desc, query folded into cols 0:16), two value slices
(128 desc each), 4x16-desc output stores. Values travel as fp8e3
(e3m4) which halves value bytes; keys/attn stay fp16.

Tensor engine: the per-slot accumulation m_j[16,450] uses only 16 of
128 PE weight columns, so 4 slots run CONCURRENTLY via column tiling:
slot j accumulates at PSUM partitions 32*(j%4)..+16 of a shared
[128,450] tile (tile_position=(base, 32*(j%4)) auto-derived). One
128-lane copy evacuates 4 slots at once to SBUF. No on-device divide:
the denominator column ships with the output and the host divides.

Device kernel per 128-token chunk k:
  scoresT [128,B] = matmul(lhsT=keyT_ext[65,128], rhs=keyT_ext[:,0:16])
      row 64 of keyT_ext is (-shift) for valid tokens / -30000 for
      padding; col 0:16 row 64 is ones -> mask+shift folded into the
      contraction (exp(-30000) == 0 exactly).
  attnT = exp(scoresT)            (ACT, PSUM -> SBUF fp16)
  per slot piece (rows a:b):
    m4[32q:32q+16] += matmul(lhsT=attnT[a:b,k], rhs=val[a:b,chunk k])
      val column 448 is 1.0 for valid rows -> denominator column.
"""

import sys

if "/opt/trn_rl_repo" not in sys.path:
    sys.path.insert(0, "/opt/trn_rl_repo")

import numpy as np
import ml_dtypes

N_CORES = 8
CHUNK = 128
ALIGN = 64          # slot capacity alignment (matmul base_partition in {0,64})
VAL_FP8 = True      # values as float8e3 (e3m4); False -> fp16
GEXP = 8            # chunks per exp batch
N_WARM = 10         # PE warm-up matmuls (HAM un-throttle)
COLT = 4            # column-tiling ways (slots per PSUM round)

LAST_RESULTS = None  # BassKernelResults of the most recent run (for test harness)


def _plan(caps):
    """Per slot, list of (chunk, a, b) partition-sliced matmul pieces."""
    pieces = []
    o = 0
    for cap in caps:
        sl = []
        lo = o
        while lo < o + cap:
            k = lo // CHUNK
            hi = min(o + cap, (k + 1) * CHUNK)
            sl.append((k, lo - k * CHUNK, hi - k * CHUNK))
            lo = hi
        pieces.append(sl)
        o += cap
    return pieces


def _build_program(caps, gpc, ca):
    from concourse import bacc, mybir
    from concourse.tile import TileContext

    vw = ca + 2  # value row width: features + denominator + pad-to-even
    sum_caps = int(sum(caps))
    n_chunks = sum_caps // CHUNK
    assert sum_caps % CHUNK == 0 and gpc % COLT == 0
    rounds = gpc // COLT
    nc = bacc.Bacc(None, target_bir_lowering=False, debug=False)

    f32 = mybir.dt.float32
    f16 = mybir.dt.float16
    bf16 = mybir.dt.bfloat16
    vdt = mybir.dt.float8e3 if VAL_FP8 else f16
    kc = 16 + sum_caps  # query cols 0:16, then keys
    keys_d = nc.dram_tensor("keysT", [65, kc], f16, kind="ExternalInput")
    vals_d = nc.dram_tensor("vals", [CHUNK, n_chunks * vw], vdt, kind="ExternalInput")
    # group-major output: group q (partitions 32q..32q+16) owns slots
    # j%COLT==q, laid out round-major within the group
    out_d = nc.dram_tensor("out", [16, gpc * vw], f16, kind="ExternalOutput")

    pieces = _plan(caps)

    def vsplit(n, parts):
        q, r = divmod(n, parts)
        out, a = [], 0
        for i in range(parts):
            b = a + q + (1 if i < r else 0)
            if b > a:
                out.append((a, b))
            a = b
        return out

    with TileContext(nc) as tc:
        with (
            tc.tile_pool(name="resid", bufs=1) as rpool,
            tc.tile_pool(name="scps", bufs=3, space="PSUM") as scpsum,
            tc.tile_pool(name="mps", bufs=3, space="PSUM") as mpsum,
            tc.tile_pool(name="wps", bufs=1, space="PSUM") as wpsum,
        ):
            ktile = rpool.tile([65, kc], f16)
            vtile = rpool.tile([CHUNK, n_chunks * vw], vdt)
            # Keys gate the whole score pipeline: split them across BOTH
            # rings as the first transfer on each (descriptor generation is
            # serial per ring at ~45ns/desc, so nothing may sit ahead of
            # them). Value slices follow, one per ring.
            vsl = vsplit(n_chunks, 2)
            kmid = 16 + vsl[0][1] * CHUNK  # keys for chunks of slice 0 + query
            nc.scalar.dma_start(out=ktile[:, :kmid], in_=keys_d[:, :kmid])
            nc.sync.dma_start(out=ktile[:, kmid:], in_=keys_d[:, kmid:])
            nc.sync.dma_start(
                out=vtile[:, : vsl[0][1] * vw], in_=vals_d[:, : vsl[0][1] * vw]
            )
            nc.scalar.dma_start(
                out=vtile[:, vsl[1][0] * vw :], in_=vals_d[:, vsl[1][0] * vw :]
            )
            obuf = rpool.tile([CHUNK, rounds * vw], f16)

            warm = rpool.tile([CHUNK, 512], bf16)
            nc.vector.memset(warm[:], 0.0)
            wps = wpsum.tile([CHUNK, 512], f32)
            for _ in range(N_WARM):
                nc.tensor.matmul(wps[:], warm[:, :CHUNK], warm[:], start=True, stop=True)

            # Eager scores + exp for every chunk; attn resident in SBUF.
            attn = rpool.tile([CHUNK, n_chunks * 16], f16)
            n_batches = -(-n_chunks // GEXP)
            for gi in range(n_batches):
                n = min(GEXP, n_chunks - gi * GEXP)
                sc = scpsum.tile([CHUNK, n * 16], f32, name=f"sc{gi}", tag="sc")
                for x in range(n):
                    t0 = 16 + (gi * GEXP + x) * CHUNK
                    nc.tensor.matmul(
                        sc[:, x * 16 : (x + 1) * 16],
                        ktile[:, t0 : t0 + CHUNK],
                        ktile[:, 0:16],
                        start=True,
                        stop=True,
                    )
                nc.scalar.activation(
                    attn[:, gi * GEXP * 16 : (gi * GEXP + n) * 16],
                    sc[:],
                    mybir.ActivationFunctionType.Exp,
                )

            # Per round: COLT slots accumulate concurrently in one PSUM tile
            # (column tiling), then one 128-lane copy evacuates all of them.
            # Output stores go out in two waves (after the half-way round and
            # at the end) so the first wave hides under remaining compute.
            engs = [nc.sync, nc.scalar]
            half = rounds // 2
            for r in range(rounds):
                m4 = mpsum.tile([CHUNK, vw], f32, name=f"m{r}", tag="m")
                for q in range(COLT):
                    j = r * COLT + q
                    sl = pieces[j]
                    for pi, (k, a, b) in enumerate(sl):
                        nc.tensor.matmul(
                            m4[32 * q : 32 * q + 16, :],
                            attn[a:b, k * 16 : (k + 1) * 16],
                            vtile[a:b, k * vw : (k + 1) * vw],
                            start=(pi == 0),
                            stop=(pi == len(sl) - 1),
                            # base partition 96 trips the auto-derive assert;
                            # positions are the operands' bases anyway
                            tile_position=(a if b - a <= 64 else 0, 32 * q),
                        )
                dst = obuf[:, r * vw : (r + 1) * vw]
                if r % 2 == 0:
                    nc.vector.tensor_copy(dst, m4[:])
                else:
                    nc.scalar.activation(
                        dst, m4[:], mybir.ActivationFunctionType.Copy
                    )
                if r == half - 1:
                    for q in range(COLT):
                        engs[q % 2].dma_start(
                            out=out_d[:, (q * rounds) * vw : (q * rounds + half) * vw],
                            in_=obuf[32 * q : 32 * q + 16, : half * vw],
                        )
            for q in range(COLT):
                engs[q % 2].dma_start(
                    out=out_d[:, (q * rounds + half) * vw : ((q + 1) * rounds) * vw],
                    in_=obuf[32 * q : 32 * q + 16, half * vw :],
                )

    nc.finalize()
    return nc


def kernel(Z_img, Z_snd, pad_idx, pad_mask, attn_dims):
    global LAST_RESULTS
    import os

    from concourse.bass_utils import run_bass_kernel_spmd

    Z_img = np.asarray(Z_img, dtype=np.float32)
    Z_snd = np.asarray(Z_snd, dtype=np.float32)
    pad_idx = np.asarray(pad_idx)
    pad_mask = np.asarray(pad_mask).astype(bool)
    A = int(attn_dims)

    B = Z_img.shape[0]
    C = Z_img.shape[1]
    CA = C - A
    G = pad_idx.shape[0]
    assert B == 16 and G % (N_CORES * COLT) == 0, (B, G)
    gpc = G // N_CORES
    rounds = gpc // COLT

    z = Z_img.reshape(B, C, -1).mean(axis=2)
    z_img, query = z[:, :CA], z[:, CA:]

    sizes = pad_mask.sum(axis=1).astype(np.int64)
    order = np.argsort(-sizes, kind="stable")  # group ids, size descending
    caps = -(-np.maximum(sizes[order[0::N_CORES]], 1) // ALIGN) * ALIGN
    if caps.sum() % CHUNK:
        caps[-1] += CHUNK - caps.sum() % CHUNK
    sum_caps = int(caps.sum())
    n_chunks = sum_caps // CHUNK
    slot_off = np.concatenate([[0], np.cumsum(caps)[:-1]]).astype(np.int64)

    q_norm_max = float(np.linalg.norm(query, axis=1).max())
    vw = CA + 2
    vdt = ml_dtypes.float8_e3m4 if VAL_FP8 else np.float16

    in_maps = []
    for c in range(N_CORES):
        keysT = np.zeros((65, 16 + sum_caps), dtype=np.float32)
        keysT[:64, 0:16] = query.T
        keysT[64, 0:16] = 1.0
        keysT[64, 16:] = -30000.0  # pad columns -> exp == 0 exactly
        vals = np.zeros((sum_caps, vw), dtype=np.float32)
        for j in range(gpc):
            g = int(order[j * N_CORES + c])
            s = int(sizes[g])
            o = int(slot_off[j])
            if s == 0:
                keysT[64, 16 + o] = 0.0
                vals[o, CA] = 1.0
                continue
            idx = pad_idx[g][pad_mask[g]]
            rows = Z_snd[idx]
            keysT[:64, 16 + o : 16 + o + s] = rows[:, CA:].T
            k_norm_max = float(np.linalg.norm(rows[:, CA:], axis=1).max())
            shift = min(q_norm_max * k_norm_max, 80.0)
            keysT[64, 16 + o : 16 + o + s] = -shift
            vals[o : o + s, :CA] = rows[:, :CA]
            vals[o : o + s, CA] = 1.0
        vimg = np.ascontiguousarray(
            vals.reshape(n_chunks, CHUNK, vw).transpose(1, 0, 2)
        ).reshape(CHUNK, n_chunks * vw).astype(vdt)
        in_maps.append({"keysT": keysT.astype(np.float16), "vals": vimg})

    nc = _build_program(caps, gpc, CA)
    trace = bool(os.environ.get("AUDIOATTN_TRACE"))
    res = run_bass_kernel_spmd(
        nc, in_maps, list(range(N_CORES)), trace=trace,
        tmpdir=os.environ.get("AUDIOATTN_TRACE_DIR") if trace else None,
    )
    LAST_RESULTS = res

    M_snd = np.empty((G, B, CA), dtype=np.float32)
    for c in range(N_CORES):
        # out layout: [16, (q * rounds + r) * vw + col], slot j = r*COLT + q
        out_c = (
            res.results[c]["out"].astype(np.float32).reshape(B, COLT, rounds, vw)
        )
        num = out_c[..., :CA]
        den = out_c[..., CA : CA + 1]
        mm = num / den  # [B, COLT, rounds, CA]
        for j in range(gpc):
            M_snd[order[j * N_CORES + c]] = mm[:, j % COLT, j // COLT]

    M_img = np.broadcast_to(z_img[None], (G, B, CA))
    return M_img, M_snd


# revision 10
# speedup vs baseline: 1.0951x; 1.0288x over previous
"""AudioAttention forward on 8 Trainium2 NeuronCores (Bass/Tile).

Reference computation (eval-mode AudioAttention):
    z      = mean_pool(Z_img)                    # [B, C]
    z_img, query = z[:, :C-A], z[:, C-A:]
    snd    = Z_snd[pad_idx]                      # [G, S, C] ragged gather
    value, key = snd[..., :C-A], snd[..., C-A:]
    scores = query @ key^T  (per group), masked softmax over S
    M_snd  = attn @ value                        # [G, B, C-A]
    M_img  = broadcast(z_img)                    # [G, B, C-A]

Sharding: groups sorted by size, dealt round-robin to 8 cores -> one
SPMD program serves all cores. Slot capacities are the per-slot max
size rounded up to 64: token chunks of 128 may span slot boundaries;
per-slot accumulation uses partition-sliced matmuls (bases in {0,64}).

DMA: each dma_start costs one descriptor per SBUF partition line and
the HWDGE ring generates descriptors serially at ~45ns each before the
doorbell, so descriptor COUNT (not bytes) sets latency. Hence: one
keys DMA (65 desc, query folded into cols 0:16), two value slices
(128 desc each), 4x16-desc output stores. Values travel as fp8e3
(e3m4) which halves value bytes; keys/attn stay fp16.

Tensor engine: the per-slot accumulation m_j[16,450] uses only 16 of
128 PE weight columns, so 4 slots run CONCURRENTLY via column tiling:
slot j accumulates at PSUM partitions 32*(j%4)..+16 of a shared
[128,450] tile (tile_position=(base, 32*(j%4)) auto-derived). One
128-lane copy evacuates 4 slots at once to SBUF. No on-device divide:
the denominator column ships with the output and the host divides.

Device kernel per 128-token chunk k:
  scoresT [128,B] = matmul(lhsT=keyT_ext[65,128], rhs=keyT_ext[:,0:16])
      row 64 of keyT_ext is (-shift) for valid tokens / -30000 for
      padding; col 0:16 row 64 is ones -> mask+shift folded into the
      contraction (exp(-30000) == 0 exactly).
  attnT = exp(scoresT)            (ACT, PSUM -> SBUF fp16)
  per slot piece (rows a:b):
    m4[32q:32q+16] += matmul(lhsT=attnT[a:b,k], rhs=val[a:b,chunk k])
      val column 448 is 1.0 for valid rows -> denominator column.
"""

import sys

if "/opt/trn_rl_repo" not in sys.path:
    sys.path.insert(0, "/opt/trn_rl_repo")

import numpy as np
import ml_dtypes

N_CORES = 8
CHUNK = 128
ALIGN = 64          # slot capacity alignment (matmul base_partition in {0,64})
VAL_FP8 = True      # values as float8e3 (e3m4); False -> fp16
GEXP = 8            # chunks per exp batch
N_WARM = 10         # PE warm-up matmuls (HAM un-throttle)
COLT = 4            # column-tiling ways (slots per PSUM round)

LAST_RESULTS = None  # BassKernelResults of the most recent run (for test harness)


def _plan(caps):
    """Per slot, list of (chunk, a, b) partition-sliced matmul pieces."""
    pieces = []
    o = 0
    for cap in caps:
        sl = []
        lo = o
        while lo < o + cap:
            k = lo // CHUNK
            hi = min(o + cap, (k + 1) * CHUNK)
            sl.append((k, lo - k * CHUNK, hi - k * CHUNK))
            lo = hi
        pieces.append(sl)
        o += cap
    return pieces


def _build_program(caps, gpc, ca):
    from concourse import bacc, mybir
    from concourse.tile import TileContext

    vw = ca + 2  # value row width: features + denominator + pad-to-even
    sum_caps = int(sum(caps))
    n_chunks = sum_caps // CHUNK
    assert sum_caps % CHUNK == 0 and gpc % COLT == 0
    rounds = gpc // COLT
    nc = bacc.Bacc(None, target_bir_lowering=False, debug=False)

    f32 = mybir.dt.float32
    f16 = mybir.dt.float16
    bf16 = mybir.dt.bfloat16
    vdt = mybir.dt.float8e3 if VAL_FP8 else f16
    kc = 16 + sum_caps  # query cols 0:16, then keys
    keys_d = nc.dram_tensor("keysT", [65, kc], f16, kind="ExternalInput")
    vals_d = nc.dram_tensor("vals", [CHUNK, n_chunks * vw], vdt, kind="ExternalInput")
    # group-major output: group q (partitions 32q..32q+16) owns slots
    # j%COLT==q, laid out round-major within the group
    out_d = nc.dram_tensor("out", [16, gpc * vw], f16, kind="ExternalOutput")

    pieces = _plan(caps)

    def vsplit(n, parts):
        q, r = divmod(n, parts)
        out, a = [], 0
        for i in range(parts):
            b = a + q + (1 if i < r else 0)
            if b > a:
                out.append((a, b))
            a = b
        return out

    with TileContext(nc) as tc:
        with (
            tc.tile_pool(name="resid", bufs=1) as rpool,
            tc.tile_pool(name="scps", bufs=3, space="PSUM") as scpsum,
            tc.tile_pool(name="mps", bufs=4, space="PSUM") as mpsum,
            tc.tile_pool(name="wps", bufs=1, space="PSUM") as wpsum,
        ):
            ktile = rpool.tile([65, kc], f16)
            vtile = rpool.tile([CHUNK, n_chunks * vw], vdt)
            # Keys gate the whole score pipeline: split them across BOTH
            # rings as the first transfer on each (descriptor generation is
            # serial per ring at ~45ns/desc, so nothing may sit ahead of
            # them). Value slices follow, one per ring.
            vsl = vsplit(n_chunks, 2)
            kmid = 16 + vsl[0][1] * CHUNK  # keys for chunks of slice 0 + query
            nc.scalar.dma_start(out=ktile[:, :kmid], in_=keys_d[:, :kmid])
            nc.sync.dma_start(out=ktile[:, kmid:], in_=keys_d[:, kmid:])
            nc.sync.dma_start(
                out=vtile[:, : vsl[0][1] * vw], in_=vals_d[:, : vsl[0][1] * vw]
            )
            nc.scalar.dma_start(
                out=vtile[:, vsl[1][0] * vw :], in_=vals_d[:, vsl[1][0] * vw :]
            )
            obuf = rpool.tile([CHUNK, rounds * vw], f16)

            warm = rpool.tile([CHUNK, 512], bf16)
            nc.vector.memset(warm[:], 0.0)
            wps = wpsum.tile([CHUNK, 512], f32)
            for _ in range(N_WARM):
                nc.tensor.matmul(wps[:], warm[:, :CHUNK], warm[:], start=True, stop=True)

            # Eager scores + exp for every chunk; attn resident in SBUF.
            attn = rpool.tile([CHUNK, n_chunks * 16], f16)
            n_batches = -(-n_chunks // GEXP)
            for gi in range(n_batches):
                n = min(GEXP, n_chunks - gi * GEXP)
                sc = scpsum.tile([CHUNK, n * 16], f32, name=f"sc{gi}", tag="sc")
                for x in range(n):
                    t0 = 16 + (gi * GEXP + x) * CHUNK
                    nc.tensor.matmul(
                        sc[:, x * 16 : (x + 1) * 16],
                        ktile[:, t0 : t0 + CHUNK],
                        ktile[:, 0:16],
                        start=True,
                        stop=True,
                    )
                nc.scalar.activation(
                    attn[:, gi * GEXP * 16 : (gi * GEXP + n) * 16],
                    sc[:],
                    mybir.ActivationFunctionType.Exp,
                )

            # Scores end ~4us before the first value slice lands; keep the PE
            # busy across that gap or HAM re-throttles it to 1.2 GHz and every
            # m-matmul runs at the cold rate.
            for _ in range(14):
                nc.tensor.matmul(wps[:], warm[:, :CHUNK], warm[:], start=True, stop=True)

            # Per round: COLT slots accumulate concurrently in one PSUM tile
            # (column tiling), then one 128-lane copy evacuates all of them.
            engs = [nc.sync, nc.scalar]
            for r in range(rounds):
                m4 = mpsum.tile([CHUNK, vw], f32, name=f"m{r}", tag="m")
                for q in range(COLT):
                    j = r * COLT + q
                    sl = pieces[j]
                    for pi, (k, a, b) in enumerate(sl):
                        nc.tensor.matmul(
                            m4[32 * q : 32 * q + 16, :],
                            attn[a:b, k * 16 : (k + 1) * 16],
                            vtile[a:b, k * vw : (k + 1) * vw],
                            start=(pi == 0),
                            stop=(pi == len(sl) - 1),
                            # base partition 96 trips the auto-derive assert;
                            # positions are the operands' bases anyway
                            tile_position=(a if b - a <= 64 else 0, 32 * q),
                        )
                dst = obuf[:, r * vw : (r + 1) * vw]
                if r % 2 == 0:
                    nc.vector.tensor_copy(dst, m4[:])
                else:
                    nc.scalar.activation(
                        dst, m4[:], mybir.ActivationFunctionType.Copy
                    )
            for q in range(COLT):
                engs[q % 2].dma_start(
                    out=out_d[:, q * rounds * vw : (q + 1) * rounds * vw],
                    in_=obuf[32 * q : 32 * q + 16, :],
                )

    nc.finalize()
    return nc


def kernel(Z_img, Z_snd, pad_idx, pad_mask, attn_dims):
    global LAST_RESULTS
    import os

    from concourse.bass_utils import run_bass_kernel_spmd

    Z_img = np.asarray(Z_img, dtype=np.float32)
    Z_snd = np.asarray(Z_snd, dtype=np.float32)
    pad_idx = np.asarray(pad_idx)
    pad_mask = np.asarray(pad_mask).astype(bool)
    A = int(attn_dims)

    B = Z_img.shape[0]
    C = Z_img.shape[1]
    CA = C - A
    G = pad_idx.shape[0]
    assert B == 16 and G % (N_CORES * COLT) == 0, (B, G)
    gpc = G // N_CORES
    rounds = gpc // COLT

    z = Z_img.reshape(B, C, -1).mean(axis=2)
    z_img, query = z[:, :CA], z[:, CA:]

    sizes = pad_mask.sum(axis=1).astype(np.int64)
    order = np.argsort(-sizes, kind="stable")  # group ids, size descending
    caps = -(-np.maximum(sizes[order[0::N_CORES]], 1) // ALIGN) * ALIGN
    if caps.sum() % CHUNK:
        caps[-1] += CHUNK - caps.sum() % CHUNK
    sum_caps = int(caps.sum())
    n_chunks = sum_caps // CHUNK
    slot_off = np.concatenate([[0], np.cumsum(caps)[:-1]]).astype(np.int64)

    q_norm_max = float(np.linalg.norm(query, axis=1).max())
    vw = CA + 2
    vdt = ml_dtypes.float8_e3m4 if VAL_FP8 else np.float16

    in_maps = []
    for c in range(N_CORES):
        keysT = np.zeros((65, 16 + sum_caps), dtype=np.float32)
        keysT[:64, 0:16] = query.T
        keysT[64, 0:16] = 1.0
        keysT[64, 16:] = -30000.0  # pad columns -> exp == 0 exactly
        vals = np.zeros((sum_caps, vw), dtype=np.float32)
        for j in range(gpc):
            g = int(order[j * N_CORES + c])
            s = int(sizes[g])
            o = int(slot_off[j])
            if s == 0:
                keysT[64, 16 + o] = 0.0
                vals[o, CA] = 1.0
                continue
            idx = pad_idx[g][pad_mask[g]]
            rows = Z_snd[idx]
            keysT[:64, 16 + o : 16 + o + s] = rows[:, CA:].T
            k_norm_max = float(np.linalg.norm(rows[:, CA:], axis=1).max())
            shift = min(q_norm_max * k_norm_max, 80.0)
            keysT[64, 16 + o : 16 + o + s] = -shift
            vals[o : o + s, :CA] = rows[:, :CA]
            vals[o : o + s, CA] = 1.0
        vimg = np.ascontiguousarray(
            vals.reshape(n_chunks, CHUNK, vw).transpose(1, 0, 2)
        ).reshape(CHUNK, n_chunks * vw).astype(vdt)
        in_maps.append({"keysT": keysT.astype(np.float16), "vals": vimg})

    nc = _build_program(caps, gpc, CA)
    trace = bool(os.environ.get("AUDIOATTN_TRACE"))
    res = run_bass_kernel_spmd(
        nc, in_maps, list(range(N_CORES)), trace=trace,
        tmpdir=os.environ.get("AUDIOATTN_TRACE_DIR") if trace else None,
    )
    LAST_RESULTS = res

    M_snd = np.empty((G, B, CA), dtype=np.float32)
    for c in range(N_CORES):
        # out layout: [16, (q * rounds + r) * vw + col], slot j = r*COLT + q
        out_c = (
            res.results[c]["out"].astype(np.float32).reshape(B, COLT, rounds, vw)
        )
        num = out_c[..., :CA]
        den = out_c[..., CA : CA + 1]
        mm = num / den  # [B, COLT, rounds, CA]
        for j in range(gpc):
            M_snd[order[j * N_CORES + c]] = mm[:, j % COLT, j // COLT]

    M_img = np.broadcast_to(z_img[None], (G, B, CA))
    return M_img, M_snd
